# revision 9
# baseline (speedup 1.0000x reference)
"""FFM layer (nn_FFM_Layer) Trainium2 Bass kernel.

Reference computation (B=4096, 13 dense fields, 26 sparse fields with vocab
1000 each, FIELD_NUM=39, K=16):

    idx        = sparse + offsets                      # [B, 26] global ids
    first      = w0 + dense @ w[:13] + sum_j w[idx]    # [B, 1]
    field_f    = einsum('bd,dfk', dense, v[:13]) + sum_j v[idx]   # [B,39,16]
    s          = field_f.sum(1)                        # [B, 16]
    second     = 0.5*(||s||^2 - sum_fk field_f^2)      # [B]
    out        = first + second[:, None]

Strategy (data-parallel over batch, 8 cores x 512 samples, no collectives):
  * Host packs an augmented table V_AUG [26013, 640] f32:
      cols [0:624]  = v.reshape(26013, 39*16)
      col  624      = w[:, 0]   (+ w0 folded into rows of sparse table 0,
                                 which every sample hits exactly once)
      cols [625:640]= 0         (pad rows to 2560 B; dma_gather requires
                                 elem_size_bytes % 256 == 0)
  * Each core dma_gathers its 512*26 rows (SWDGE mlp ucode). The SWDGE
    stream is the roofline: ~34.1 MB/core at ~383 GB/s effective = ~89 us.
    Schedule is tuned so the stream starts early and ends with a tiny call:
      - idx table is stored [16, cols] in DRAM (ucode reads 16 idx
        partitions x 8 Q7-core replicas); one 16-packet DMA + 3 on-chip
        partition-doubling copies replaces a 512-packet load, moving the
        first descriptor ~8 us earlier.
      - call sizes [2,6,6,6,6]/[7,7,6,6]/[7,7,6,6]/[7,7,6,5,1] alternate
        between the 2 SWDGE queues -> exactly 52/52 nf-units per queue
        (stream end is set by the fuller queue).
      - the global last call is nf=1: no fold needed, fld = g + psum, so
        the post-stream tail is just the FM epilogue chain.
  * DVE folds each call's cols pairwise; PE accumulates folded cols into
    the chunk's PSUM chain seeded by the dense [13,128]x[13,640] matmul.
  * FM identity epilogue per chunk: ACT Square+accum_out for both norms,
    one strided DVE tensor_reduce over [128,16,39] for the s-sum.
  * Result [128,4] is PE-transposed to [4,128] so the final store is 4
    packets instead of 128 on the slow hardware-dynamic queue.
"""

import sys

if "/opt/trn_rl_repo" not in sys.path:
    sys.path.insert(0, "/opt/trn_rl_repo")

import numpy as np

import concourse.bacc as bacc
import concourse.bass as bass
import concourse.tile as tile
from concourse import mybir
from concourse.bass_utils import run_bass_kernel_spmd

# Problem constants (hardcoded per harness contract)
B = 4096
N_DENSE = 13
N_SPARSE = 26
FEAT_PER_SPARSE = 1000
FIELD_NUM = 39
FEATURE_NUM = 26013
K = 16
N_CORES = 8
BC = B // N_CORES          # 512 samples per core
ROW = 640                  # padded row: 624 v + 1 w + 15 zeros (2560 B)
VCOLS = FIELD_NUM * K      # 624
P = 128
SCHUNKS = BC // P          # 4 sample chunks of 128 per core
# per-chunk gather calls: field groups (sum 26 each).  Queue = call_no % 2
# gives 52/52 nf-units per queue; last global call is nf=1 (short tail).
FGROUPS_ALL = [
    [2, 6, 6, 6, 6],
    [7, 7, 6, 6],
    [7, 7, 6, 6],
    [7, 7, 6, 5, 1],
]
IDX_COLS_SC = N_SPARSE * P // 16   # 208 idx cols per sample chunk
IDX_COLS = SCHUNKS * IDX_COLS_SC   # 832

F32 = mybir.dt.float32
I16 = mybir.dt.int16


def build_program():
    """Build + compile the single-core SPMD bass program."""
    nc = bacc.Bacc("TRN2", target_bir_lowering=False, debug=False,
                   num_swdge_queues=2)

    vaug_t = nc.dram_tensor("vaug", [FEATURE_NUM, ROW], F32, kind="ExternalInput")
    dense_t = nc.dram_tensor("dense_t", [N_DENSE, BC], F32, kind="ExternalInput")
    idxs_t = nc.dram_tensor("idxs", [32, IDX_COLS], I16, kind="ExternalInput")
    ident_t = nc.dram_tensor("ident", [P, P], F32, kind="ExternalInput")
    out_t = nc.dram_tensor("out", [SCHUNKS, P], F32, kind="ExternalOutput")

    with tile.TileContext(nc) as tc:
        with (
            tc.tile_pool(name="main", bufs=1) as main,
            tc.tile_pool(name="gath", bufs=8) as gath,
            tc.tile_pool(name="fold", bufs=3) as fold,
            tc.tile_pool(name="small", bufs=2) as small,
            tc.tile_pool(name="psum", bufs=3, space="PSUM") as psum,
        ):
            # Warm-up gather: the first SWDGE call pays a ~5 us one-time
            # init (ucode load / queue priming); trigger it immediately with
            # a self-made zero idx (gpsimd memset, no DMA dependency) so the
            # real stream isn't delayed behind it.
            widx = main.tile([P, 1], I16)
            nc.gpsimd.memset(widx[:], 0)
            wg = gath.tile([P, 1, ROW], F32, tag="wg", bufs=1)
            nc.gpsimd.dma_gather(
                wg[:, :, :], vaug_t[:], widx[:], 16, 16, ROW,
                single_packet=False, queue_num=0,
            )

            # idx fast path: 32-partition DMA (2 of the 8 16-row replicas the
            # SWDGE ucode reads) + 3 copies at legal 32-partition offsets
            idx128 = main.tile([P, IDX_COLS], I16)
            nc.sync.dma_start(idx128[0:32, :], idxs_t[:])
            nc.vector.tensor_copy(idx128[32:64, :], idx128[0:32, :])
            nc.vector.tensor_copy(idx128[64:96, :], idx128[0:32, :])
            nc.vector.tensor_copy(idx128[96:128, :], idx128[0:32, :])

            vaug13 = main.tile([N_DENSE, ROW], F32)
            nc.sync.dma_start(vaug13[:], vaug_t[0:N_DENSE, :])
            dt_sb = main.tile([N_DENSE, BC], F32)
            nc.sync.dma_start(dt_sb[:], dense_t[:])
            ident = main.tile([P, P], F32)
            nc.sync.dma_start(ident[:], ident_t[:])

            res = main.tile([P, SCHUNKS], F32)

            # warm-up was queue 0; keep the dma_gather -> queue sequence
            # perfectly alternating so per-queue semaphores stay aligned
            call_no = 1
            for c in range(SCHUNKS):
                fgroups = FGROUPS_ALL[c]
                # dense part seeds this chunk's PSUM accumulation chain
                ps = psum.tile([P, ROW], F32, tag="ps")
                lhs_d = dt_sb[:, c * P : (c + 1) * P]
                nc.tensor.matmul(out=ps[:, 0:512], lhsT=lhs_d,
                                 rhs=vaug13[:, 0:512], start=True, stop=False)
                nc.tensor.matmul(out=ps[:, 512:ROW], lhsT=lhs_d,
                                 rhs=vaug13[:, 512:ROW], start=True, stop=False)

                def fold_call(eng, g, nf):
                    """Pairwise-tree fold of the call's nf cols -> [P,ROW]."""
                    if nf == 1:
                        return g[:, 0, :]
                    t2 = small.tile([P, ROW], F32, tag="t2", bufs=4,
                                    name=f"t2_{c}")
                    if nf >= 6:
                        t1 = fold.tile([P, 3, ROW], F32, tag="t1",
                                       name=f"t1_{c}")
                        eng.tensor_add(t1[:], g[:, 0:3, :], g[:, 3:6, :])
                        eng.tensor_add(t2[:], t1[:, 0, :], t1[:, 1, :])
                        eng.tensor_add(t2[:], t2[:], t1[:, 2, :])
                        if nf == 7:
                            eng.tensor_add(t2[:], t2[:], g[:, 6, :])
                    elif nf in (4, 5):
                        t1 = fold.tile([P, 3, ROW], F32, tag="t1",
                                       name=f"t1_{c}")
                        eng.tensor_add(t1[:, 0:2, :], g[:, 0:2, :], g[:, 2:4, :])
                        eng.tensor_add(t2[:], t1[:, 0, :], t1[:, 1, :])
                        if nf == 5:
                            eng.tensor_add(t2[:], t2[:], g[:, 4, :])
                    else:  # nf == 2
                        eng.tensor_add(t2[:], g[:, 0, :], g[:, 1, :])
                    return t2

                def accum(t2, stop):
                    nc.tensor.matmul(out=ps[:, 0:512], lhsT=ident[:],
                                     rhs=t2[:, 0:512], start=False, stop=stop)
                    nc.tensor.matmul(out=ps[:, 512:ROW], lhsT=ident[:],
                                     rhs=t2[:, 512:ROW], start=False, stop=stop)

                icol = c * IDX_COLS_SC
                if c < SCHUNKS - 1:
                    # steady-state: gather -> fold (DVE) -> PE accumulate
                    last_data = None
                    for gi, nf in enumerate(fgroups):
                        n_idx = nf * P
                        g = gath.tile([P, 7, ROW], F32, tag="g")
                        nc.gpsimd.dma_gather(
                            g[:, :nf, :], vaug_t[:],
                            idx128[:, icol : icol + n_idx // 16],
                            n_idx, n_idx, ROW,
                            single_packet=False, queue_num=call_no % 2,
                        )
                        icol += n_idx // 16
                        call_no += 1
                        t2 = fold_call(nc.vector, g, nf)
                        if gi == len(fgroups) - 1:
                            last_data = t2
                        else:
                            accum(t2, stop=gi == len(fgroups) - 2)
                else:
                    # tail chunk: issue ALL gathers first (desc-gen must not
                    # sit behind pool folds in the gpsimd queue), then fold
                    # with the second call on the now-idle Pool engine.
                    gs = []
                    for nf in fgroups:
                        n_idx = nf * P
                        g = gath.tile([P, 7, ROW], F32, tag="g")
                        nc.gpsimd.dma_gather(
                            g[:, :nf, :], vaug_t[:],
                            idx128[:, icol : icol + n_idx // 16],
                            n_idx, n_idx, ROW,
                            single_packet=False, queue_num=call_no % 2,
                        )
                        icol += n_idx // 16
                        call_no += 1
                        gs.append(g)
                    engines = [nc.vector, nc.gpsimd, nc.vector, nc.vector]
                    for gi, nf in enumerate(fgroups[:-1]):
                        t2 = fold_call(engines[gi], gs[gi], nf)
                        accum(t2, stop=gi == len(fgroups) - 2)
                    last_data = gs[-1][:, 0, :]   # nf == 1: no fold needed

                # fld = psum chain + last col (single PSUM read on DVE)
                fld = fold.tile([P, ROW], F32, tag="fld")
                nc.vector.tensor_add(fld[:], last_data[:], ps[:])

                # --- FM identity epilogue for this chunk ---
                blk = fld[:, 0:VCOLS]             # [128, 624] = field_f
                sq = fold.tile([P, VCOLS], F32, tag="sq")
                q = small.tile([P, 1], F32, tag="q")
                nc.scalar.activation(
                    sq[:], blk, mybir.ActivationFunctionType.Square,
                    accum_out=q[:],
                )
                # s = sum over the 39 fields: strided reduce of [128,16,39]
                st = fold.tile([P, 16], F32, tag="st")
                blk_kf = blk.rearrange("p (f k) -> p k f", k=16)
                nc.vector.tensor_reduce(
                    out=st[:], in_=blk_kf, op=mybir.AluOpType.add,
                    axis=mybir.AxisListType.X,
                )
                s2 = small.tile([P, 16], F32, tag="s2")
                snorm = small.tile([P, 1], F32, tag="snorm")
                nc.scalar.activation(
                    s2[:], st[:], mybir.ActivationFunctionType.Square,
                    accum_out=snorm[:],
                )
                diff = small.tile([P, 1], F32, tag="diff")
                nc.vector.tensor_tensor(
                    out=diff[:], in0=snorm[:], in1=q[:],
                    op=mybir.AluOpType.subtract,
                )
                # out = 0.5*diff + (w-sum incl. w0 and dense first-order)
                nc.scalar.activation(
                    res[:, c : c + 1],
                    diff[:],
                    mybir.ActivationFunctionType.Identity,
                    bias=fld[:, VCOLS : VCOLS + 1],
                    scale=0.5,
                )

            # transpose [128,4] -> [4,128] on PE so the store is 4 packets
            pst = psum.tile([SCHUNKS, P], F32, tag="pst", bufs=1)
            nc.tensor.transpose(pst[:], res[:], ident[:])
            res_t = main.tile([SCHUNKS, P], F32)
            nc.scalar.copy(res_t[:], pst[:])
            nc.sync.dma_start(out_t[:], res_t[:])

    nc.compile()
    return nc


def prep_inputs(dense_inputs, sparse_inputs, w0, w, v):
    """Host-side shard/pack: build per-core in_maps."""
    dense = np.asarray(dense_inputs, np.float32)
    sparse = np.asarray(sparse_inputs)
    w0 = np.asarray(w0, np.float32)
    w = np.asarray(w, np.float32)
    v = np.asarray(v, np.float32)

    vaug = np.zeros((FEATURE_NUM, ROW), np.float32)
    vaug[:, :VCOLS] = v.reshape(FEATURE_NUM, VCOLS)
    vaug[:, VCOLS] = w[:, 0]
    # fold w0 into sparse table 0 (each sample hits it exactly once)
    vaug[N_DENSE : N_DENSE + FEAT_PER_SPARSE, VCOLS] += w0[0]

    offs = N_DENSE + FEAT_PER_SPARSE * np.arange(N_SPARSE, dtype=np.int64)
    gidx = (sparse.astype(np.int64) + offs[None, :]).astype(np.int16)  # [B, 26]

    in_maps = []
    for core in range(N_CORES):
        sl = slice(core * BC, (core + 1) * BC)
        dt = np.ascontiguousarray(dense[sl].T)          # [13, 512]
        idxc = gidx[sl]                                 # [512, 26]
        buf = np.zeros((16, IDX_COLS), np.int16)  # wrapped 16-row pattern
        off_c = 0
        for c in range(SCHUNKS):
            rows = idxc[c * P : (c + 1) * P]            # [128, 26]
            fbase = 0
            for nf in FGROUPS_ALL[c]:
                n = nf * P
                # call order: i = f_local*128 + p  ->  row idx[p, fbase+f]
                seg = np.ascontiguousarray(
                    rows[:, fbase : fbase + nf].T
                ).reshape(-1)                           # [nf*128]
                buf[:, off_c : off_c + n // 16] = seg.reshape(n // 16, 16).T
                fbase += nf
                off_c += n // 16
        in_maps.append({"vaug": vaug, "dense_t": dt,
                        "idxs": np.tile(buf, (2, 1)),
                        "ident": np.eye(P, dtype=np.float32)})
    return in_maps


_NC_CACHE = None


def kernel(dense_inputs, sparse_inputs, w0, w, v):
    global _NC_CACHE
    if _NC_CACHE is None:
        _NC_CACHE = build_program()
    nc = _NC_CACHE
    in_maps = prep_inputs(dense_inputs, sparse_inputs, w0, w, v)
    res = run_bass_kernel_spmd(nc, in_maps, core_ids=list(range(N_CORES)))
    outs = []
    for r in res.results:
        o = r["out"]                                    # [4, 128]
        outs.append(np.ascontiguousarray(o).reshape(BC, 1))
    return np.concatenate(outs, axis=0).astype(np.float32)


# revision 10
# speedup vs baseline: 1.0437x; 1.0437x over previous
"""FFM layer (nn_FFM_Layer) Trainium2 Bass kernel.

Reference computation (B=4096, 13 dense fields, 26 sparse fields with vocab
1000 each, FIELD_NUM=39, K=16):

    idx        = sparse + offsets                      # [B, 26] global ids
    first      = w0 + dense @ w[:13] + sum_j w[idx]    # [B, 1]
    field_f    = einsum('bd,dfk', dense, v[:13]) + sum_j v[idx]   # [B,39,16]
    s          = field_f.sum(1)                        # [B, 16]
    second     = 0.5*(||s||^2 - sum_fk field_f^2)      # [B]
    out        = first + second[:, None]

Strategy (data-parallel over batch, 8 cores x 512 samples, no collectives):
  * Host packs an augmented table V_AUG [26013, 640] f32:
      cols [0:624]  = v.reshape(26013, 39*16)
      col  624      = w[:, 0]   (+ w0 folded into rows of sparse table 0,
                                 which every sample hits exactly once)
      cols [625:640]= 0         (pad rows to 2560 B; dma_gather requires
                                 elem_size_bytes % 256 == 0)
  * Each core dma_gathers its 512*26 rows (SWDGE mlp ucode). The SWDGE
    stream is the roofline: ~34.1 MB/core at ~383 GB/s effective = ~89 us.
    Schedule tuning:
      - idx table is stored [32, cols] in DRAM (the ucode reads 8 16-row
        replicas); four 32-packet DMAs into partition quarters replace the
        512-packet load of a [128, cols] table, so desc-gen starts earlier.
      - calls are kept small (nf<=5 fields x 128 samples): same-queue
        arrival spacing ~7 us with a ~2 us DVE fold per call keeps the
        fold pipeline from ever backlogging behind the stream.
      - call sizes [2,4x6]/[4x5,3,3]/[4x5,3,3]/[5,4x5,1] on alternating
        queues give exactly 52/52 nf-units per queue (stream end is set
        by the fuller queue); the global last call is nf=1 so only the
        FM epilogue chain remains after the last byte lands.
  * DVE folds each call's cols pairwise; PE accumulates folded cols into
    the chunk's PSUM chain seeded by the dense [13,128]x[13,640] matmul.
  * FM identity epilogue per chunk: ACT Square+accum_out for both norms,
    one strided DVE tensor_reduce over [128,16,39] for the s-sum.
  * Result [128,4] is PE-transposed to [4,128] so the final store is 4
    packets instead of 128 on the slow hardware-dynamic queue.
"""

import sys

if "/opt/trn_rl_repo" not in sys.path:
    sys.path.insert(0, "/opt/trn_rl_repo")

import numpy as np

import concourse.bacc as bacc
import concourse.bass as bass
import concourse.tile as tile
from concourse import mybir
from concourse.bass_utils import run_bass_kernel_spmd

# Problem constants (hardcoded per harness contract)
B = 4096
N_DENSE = 13
N_SPARSE = 26
FEAT_PER_SPARSE = 1000
FIELD_NUM = 39
FEATURE_NUM = 26013
K = 16
N_CORES = 8
BC = B // N_CORES          # 512 samples per core
ROW = 640                  # padded row: 624 v + 1 w + 15 zeros (2560 B)
VCOLS = FIELD_NUM * K      # 624
P = 128
SCHUNKS = BC // P          # 4 sample chunks of 128 per core
GMAX = 5                   # max fields per gather call
# per-chunk gather calls: field groups (sum 26 each).  Queue = call_no % 2
# gives 52/52 nf-units per queue; last global call is nf=1 (short tail).
FGROUPS_ALL = [
    [2, 4, 4, 4, 4, 4, 4],
    [4, 4, 4, 4, 4, 3, 3],
    [4, 4, 4, 4, 4, 3, 3],
    [5, 4, 4, 4, 4, 4, 1],
]
IDX_COLS_SC = N_SPARSE * P // 16   # 208 idx cols per sample chunk
IDX_COLS = SCHUNKS * IDX_COLS_SC   # 832

F32 = mybir.dt.float32
I16 = mybir.dt.int16


def build_program():
    """Build + compile the single-core SPMD bass program."""
    nc = bacc.Bacc("TRN2", target_bir_lowering=False, debug=False,
                   num_swdge_queues=2)

    vaug_t = nc.dram_tensor("vaug", [FEATURE_NUM, ROW], F32, kind="ExternalInput")
    dense_t = nc.dram_tensor("dense_t", [N_DENSE, BC], F32, kind="ExternalInput")
    idxs_t = nc.dram_tensor("idxs", [32, IDX_COLS], I16, kind="ExternalInput")
    ident_t = nc.dram_tensor("ident", [P, P], F32, kind="ExternalInput")
    out_t = nc.dram_tensor("out", [SCHUNKS, P], F32, kind="ExternalOutput")

    with tile.TileContext(nc) as tc:
        with (
            tc.tile_pool(name="main", bufs=1) as main,
            tc.tile_pool(name="gath", bufs=10) as gath,
            tc.tile_pool(name="fold", bufs=3) as fold,
            tc.tile_pool(name="small", bufs=2) as small,
            tc.tile_pool(name="psum", bufs=3, space="PSUM") as psum,
        ):
            # idx fast path: 4 x 32-packet DMAs fill the 8 16-row replicas
            # the SWDGE ucode reads (pure DMA deps -> desc-gen starts early)
            idx128 = main.tile([P, IDX_COLS], I16)
            for qtr in range(4):
                nc.sync.dma_start(idx128[32 * qtr : 32 * (qtr + 1), :],
                                  idxs_t[:])

            vaug13 = main.tile([N_DENSE, ROW], F32)
            nc.sync.dma_start(vaug13[:], vaug_t[0:N_DENSE, :])
            dt_sb = main.tile([N_DENSE, BC], F32)
            nc.sync.dma_start(dt_sb[:], dense_t[:])
            ident = main.tile([P, P], F32)
            nc.sync.dma_start(ident[:], ident_t[:])

            res = main.tile([P, SCHUNKS], F32)

            call_no = 0
            for c in range(SCHUNKS):
                fgroups = FGROUPS_ALL[c]
                # dense part seeds this chunk's PSUM accumulation chain
                ps = psum.tile([P, ROW], F32, tag="ps")
                lhs_d = dt_sb[:, c * P : (c + 1) * P]
                nc.tensor.matmul(out=ps[:, 0:512], lhsT=lhs_d,
                                 rhs=vaug13[:, 0:512], start=True, stop=False)
                nc.tensor.matmul(out=ps[:, 512:ROW], lhsT=lhs_d,
                                 rhs=vaug13[:, 512:ROW], start=True, stop=False)

                icol = c * IDX_COLS_SC
                last_data = None     # [P, ROW] view with the last call's fold
                for gi, nf in enumerate(fgroups):
                    n_idx = nf * P
                    g = gath.tile([P, GMAX, ROW], F32, tag="g")
                    nc.gpsimd.dma_gather(
                        g[:, :nf, :],
                        vaug_t[:],
                        idx128[:, icol : icol + n_idx // 16],
                        n_idx,
                        n_idx,
                        ROW,
                        single_packet=False,
                        queue_num=call_no % 2,
                    )
                    icol += n_idx // 16
                    call_no += 1
                    # DVE fold: call's nf cols -> t2 (pairwise tree)
                    if nf == 1:
                        t2 = g[:, 0, :]
                    else:
                        t2 = small.tile([P, ROW], F32, tag="t2", bufs=4)
                        if nf >= 4:
                            t1 = fold.tile([P, 2, ROW], F32, tag="t1")
                            nc.vector.tensor_add(t1[:], g[:, 0:2, :],
                                                 g[:, 2:4, :])
                            nc.vector.tensor_add(t2[:], t1[:, 0, :],
                                                 t1[:, 1, :])
                            if nf == 5:
                                nc.vector.tensor_add(t2[:], t2[:], g[:, 4, :])
                        elif nf == 3:
                            nc.vector.tensor_add(t2[:], g[:, 0, :], g[:, 1, :])
                            nc.vector.tensor_add(t2[:], t2[:], g[:, 2, :])
                        else:  # nf == 2
                            nc.vector.tensor_add(t2[:], g[:, 0, :], g[:, 1, :])
                    if gi == len(fgroups) - 1:
                        last_data = t2
                    else:
                        # PE accumulates fold cols into the psum chain; the
                        # final col goes via DVE to keep PE out of the tail
                        stop = gi == len(fgroups) - 2
                        nc.tensor.matmul(out=ps[:, 0:512], lhsT=ident[:],
                                         rhs=t2[:, 0:512],
                                         start=False, stop=stop)
                        nc.tensor.matmul(out=ps[:, 512:ROW], lhsT=ident[:],
                                         rhs=t2[:, 512:ROW],
                                         start=False, stop=stop)

                # fld = psum chain + last col (single PSUM read on DVE)
                fld = fold.tile([P, ROW], F32, tag="fld")
                nc.vector.tensor_add(fld[:], last_data[:], ps[:])

                # --- FM identity epilogue for this chunk ---
                blk = fld[:, 0:VCOLS]             # [128, 624] = field_f
                sq = fold.tile([P, VCOLS], F32, tag="sq")
                q = small.tile([P, 1], F32, tag="q")
                nc.scalar.activation(
                    sq[:], blk, mybir.ActivationFunctionType.Square,
                    accum_out=q[:],
                )
                # s = sum over the 39 fields: strided reduce of [128,16,39]
                st = fold.tile([P, 16], F32, tag="st")
                blk_kf = blk.rearrange("p (f k) -> p k f", k=16)
                nc.vector.tensor_reduce(
                    out=st[:], in_=blk_kf, op=mybir.AluOpType.add,
                    axis=mybir.AxisListType.X,
                )
                s2 = small.tile([P, 16], F32, tag="s2")
                snorm = small.tile([P, 1], F32, tag="snorm")
                nc.scalar.activation(
                    s2[:], st[:], mybir.ActivationFunctionType.Square,
                    accum_out=snorm[:],
                )
                diff = small.tile([P, 1], F32, tag="diff")
                nc.vector.tensor_tensor(
                    out=diff[:], in0=snorm[:], in1=q[:],
                    op=mybir.AluOpType.subtract,
                )
                # out = 0.5*diff + (w-sum incl. w0 and dense first-order)
                nc.scalar.activation(
                    res[:, c : c + 1],
                    diff[:],
                    mybir.ActivationFunctionType.Identity,
                    bias=fld[:, VCOLS : VCOLS + 1],
                    scale=0.5,
                )

            # transpose [128,4] -> [4,128] on PE so the store is 4 packets
            pst = psum.tile([SCHUNKS, P], F32, tag="pst", bufs=1)
            nc.tensor.transpose(pst[:], res[:], ident[:])
            res_t = main.tile([SCHUNKS, P], F32)
            nc.scalar.copy(res_t[:], pst[:])
            nc.sync.dma_start(out_t[:], res_t[:])

    nc.compile()
    return nc


def prep_inputs(dense_inputs, sparse_inputs, w0, w, v):
    """Host-side shard/pack: build per-core in_maps."""
    dense = np.asarray(dense_inputs, np.float32)
    sparse = np.asarray(sparse_inputs)
    w0 = np.asarray(w0, np.float32)
    w = np.asarray(w, np.float32)
    v = np.asarray(v, np.float32)

    vaug = np.zeros((FEATURE_NUM, ROW), np.float32)
    vaug[:, :VCOLS] = v.reshape(FEATURE_NUM, VCOLS)
    vaug[:, VCOLS] = w[:, 0]
    # fold w0 into sparse table 0 (each sample hits it exactly once)
    vaug[N_DENSE : N_DENSE + FEAT_PER_SPARSE, VCOLS] += w0[0]

    offs = N_DENSE + FEAT_PER_SPARSE * np.arange(N_SPARSE, dtype=np.int64)
    gidx = (sparse.astype(np.int64) + offs[None, :]).astype(np.int16)  # [B, 26]

    in_maps = []
    for core in range(N_CORES):
        sl = slice(core * BC, (core + 1) * BC)
        dt = np.ascontiguousarray(dense[sl].T)          # [13, 512]
        idxc = gidx[sl]                                 # [512, 26]
        buf = np.zeros((16, IDX_COLS), np.int16)        # wrapped 16-row pattern
        off_c = 0
        for c in range(SCHUNKS):
            rows = idxc[c * P : (c + 1) * P]            # [128, 26]
            fbase = 0
            for nf in FGROUPS_ALL[c]:
                n = nf * P
                # call order: i = f_local*128 + p  ->  row idx[p, fbase+f]
                seg = np.ascontiguousarray(
                    rows[:, fbase : fbase + nf].T
                ).reshape(-1)                           # [nf*128]
                buf[:, off_c : off_c + n // 16] = seg.reshape(n // 16, 16).T
                fbase += nf
                off_c += n // 16
        in_maps.append({"vaug": vaug, "dense_t": dt,
                        "idxs": np.tile(buf, (2, 1)),
                        "ident": np.eye(P, dtype=np.float32)})
    return in_maps


_NC_CACHE = None


def kernel(dense_inputs, sparse_inputs, w0, w, v):
    global _NC_CACHE
    if _NC_CACHE is None:
        _NC_CACHE = build_program()
    nc = _NC_CACHE
    in_maps = prep_inputs(dense_inputs, sparse_inputs, w0, w, v)
    res = run_bass_kernel_spmd(nc, in_maps, core_ids=list(range(N_CORES)))
    outs = []
    for r in res.results:
        o = r["out"]                                    # [4, 128]
        outs.append(np.ascontiguousarray(o).reshape(BC, 1))
    return np.concatenate(outs, axis=0).astype(np.float32)


# revision 13
# speedup vs baseline: 1.1244x; 1.0773x over previous
"""FFM layer (nn_FFM_Layer) Trainium2 Bass kernel.

Reference computation (B=4096, 13 dense fields, 26 sparse fields with vocab
1000 each, FIELD_NUM=39, K=16):

    idx        = sparse + offsets                      # [B, 26] global ids
    first      = w0 + dense @ w[:13] + sum_j w[idx]    # [B, 1]
    field_f    = einsum('bd,dfk', dense, v[:13]) + sum_j v[idx]   # [B,39,16]
    s          = field_f.sum(1)                        # [B, 16]
    second     = 0.5*(||s||^2 - sum_fk field_f^2)      # [B]
    out        = first + second[:, None]

Strategy (data-parallel over batch, 8 cores x 512 samples, no collectives):
  * Host packs an augmented table V_AUG [26013, 640] f32:
      cols [0:624]  = v.reshape(26013, 39*16)
      col  624      = w[:, 0]   (+ w0 folded into rows of sparse table 0)
      cols [625:640]= 0         (2560 B rows; dma_gather needs %256==0)
  * Each core dma_gathers its 512*26 rows (SWDGE).  Trace-derived model:
    the SWDGE ring holds ~128 16-row entries per queue (~5.2 MB) and
    desc-gen is ring-backpressured, so
        stream_end ~= gen_end + decay,
    where the decay is the serial drain of the last ring blocks (16-row
    entries drain in 8-entry blocks per DMA engine at ~21.5 GB/s: a full
    2560B block is 0.33 MB ~= 15 us).  Schedule consequences:
      - gather calls alternate queues in EQUAL-SIZE PAIRS: the gpsimd
        engine can run one desc-gen ucode instance per queue concurrently,
        and a pair runs at max(a, b) - unequal pairs waste gen overlap.
        Pairing also balances queue bytes exactly (stream end is set by
        the fuller queue).
      - the final 128 rows per queue are gathered as 2 half-row calls
        (elem 1280 B, elem_step 640 f32): the last ring blocks carry half
        the bytes, halving the post-gen decay.
      - idx table is stored [32, cols] in DRAM (the ucode reads 8 16-row
        replicas); 4x 32-packet DMAs into partition quarters replace a
        512-packet load, so desc-gen starts earlier.
  * DVE folds each call's cols pairwise; PE accumulates folded cols into
    the chunk's PSUM chain seeded by the dense [13,128]x[13,640] matmul.
  * FM identity epilogue per chunk: ACT Square+accum_out for both norms,
    one strided DVE tensor_reduce over [128,16,39] for the s-sum.
  * Result [128,4] is PE-transposed to [4,128] so the final store is 4
    packets instead of 128 on the slow hardware-dynamic queue.
"""

import sys

if "/opt/trn_rl_repo" not in sys.path:
    sys.path.insert(0, "/opt/trn_rl_repo")

import numpy as np

import concourse.bacc as bacc
import concourse.bass as bass
import concourse.tile as tile
from concourse import mybir
from concourse.bass_utils import run_bass_kernel_spmd

# Problem constants (hardcoded per harness contract)
B = 4096
N_DENSE = 13
N_SPARSE = 26
FEAT_PER_SPARSE = 1000
FIELD_NUM = 39
FEATURE_NUM = 26013
K = 16
N_CORES = 8
BC = B // N_CORES          # 512 samples per core
ROW = 640                  # padded row: 624 v + 1 w + 15 zeros (2560 B)
HROW = ROW // 2            # 320 f32 = 1280 B (taper half-rows)
VCOLS = FIELD_NUM * K      # 624
P = 128
SCHUNKS = BC // P          # 4 sample chunks of 128 per core
GMAX = 7                   # max fields per gather call
# per-chunk gather calls (sum 26 each), queue = call_no % 2; equal pairs.
# chunk 3 ends [.., 2, 2, 1, 1]; the trailing (1,1) pair is gathered as
# half-row taper calls (see build_program).
FGROUPS_ALL = [
    [2, 2, 6, 6, 5, 5],
    [7, 7, 6, 6],
    [7, 7, 6, 6],
    [6, 6, 4, 4, 2, 2, 1, 1],
]
IDX_COLS_SC = N_SPARSE * P // 16   # 208 idx cols per sample chunk
IDX_COLS = SCHUNKS * IDX_COLS_SC   # 832

F32 = mybir.dt.float32
I16 = mybir.dt.int16


def build_program():
    """Build + compile the single-core SPMD bass program."""
    nc = bacc.Bacc("TRN2", target_bir_lowering=False, debug=False,
                   num_swdge_queues=2)

    vaug_t = nc.dram_tensor("vaug", [FEATURE_NUM, ROW], F32, kind="ExternalInput")
    dense_t = nc.dram_tensor("dense_t", [N_DENSE, BC], F32, kind="ExternalInput")
    idxs_t = nc.dram_tensor("idxs", [32, IDX_COLS], I16, kind="ExternalInput")
    ident_t = nc.dram_tensor("ident", [P, P], F32, kind="ExternalInput")
    out_t = nc.dram_tensor("out", [SCHUNKS, P], F32, kind="ExternalOutput")

    with tile.TileContext(nc) as tc:
        with (
            tc.tile_pool(name="main", bufs=1) as main,
            tc.tile_pool(name="gath", bufs=8) as gath,
            tc.tile_pool(name="fold", bufs=3) as fold,
            tc.tile_pool(name="small", bufs=2) as small,
            tc.tile_pool(name="psum", bufs=3, space="PSUM") as psum,
        ):
            # idx fast path: 4 x 32-packet DMAs fill the 8 16-row replicas
            # the SWDGE ucode reads (pure DMA deps -> desc-gen starts early)
            idx128 = main.tile([P, IDX_COLS], I16)
            for qtr in range(4):
                nc.sync.dma_start(idx128[32 * qtr : 32 * (qtr + 1), :],
                                  idxs_t[:])

            vaug13 = main.tile([N_DENSE, ROW], F32)
            nc.sync.dma_start(vaug13[:], vaug_t[0:N_DENSE, :])
            dt_sb = main.tile([N_DENSE, BC], F32)
            nc.sync.dma_start(dt_sb[:], dense_t[:])
            ident = main.tile([P, P], F32)
            nc.sync.dma_start(ident[:], ident_t[:])

            res = main.tile([P, SCHUNKS], F32)

            def fold_call(g, nf, c):
                """Pairwise-tree fold of a call's nf cols -> [P, ROW] view."""
                if nf == 1:
                    return g[:, 0, :]
                t2 = small.tile([P, ROW], F32, tag="t2", bufs=4, name=f"t2_{c}")
                if nf >= 4:
                    t1 = fold.tile([P, 2, ROW], F32, tag="t1", name=f"t1_{c}")
                    nc.vector.tensor_add(t1[:], g[:, 0:2, :], g[:, 2:4, :])
                    nc.vector.tensor_add(t2[:], t1[:, 0, :], t1[:, 1, :])
                    for x in range(4, nf):
                        nc.vector.tensor_add(t2[:], t2[:], g[:, x, :])
                elif nf == 3:
                    nc.vector.tensor_add(t2[:], g[:, 0, :], g[:, 1, :])
                    nc.vector.tensor_add(t2[:], t2[:], g[:, 2, :])
                else:  # nf == 2
                    nc.vector.tensor_add(t2[:], g[:, 0, :], g[:, 1, :])
                return t2

            call_no = 0
            for c in range(SCHUNKS):
                fgroups = FGROUPS_ALL[c]
                taper = c == SCHUNKS - 1
                ncalls = len(fgroups) - (2 if taper else 0)
                # dense part seeds this chunk's PSUM accumulation chain
                ps = psum.tile([P, ROW], F32, tag="ps")
                lhs_d = dt_sb[:, c * P : (c + 1) * P]
                nc.tensor.matmul(out=ps[:, 0:512], lhsT=lhs_d,
                                 rhs=vaug13[:, 0:512], start=True, stop=False)
                nc.tensor.matmul(out=ps[:, 512:ROW], lhsT=lhs_d,
                                 rhs=vaug13[:, 512:ROW], start=True, stop=False)

                icol = c * IDX_COLS_SC
                last_data = None
                for gi in range(ncalls):
                    nf = fgroups[gi]
                    n_idx = nf * P
                    g = gath.tile([P, GMAX, ROW], F32, tag="g")
                    nc.gpsimd.dma_gather(
                        g[:, :nf, :],
                        vaug_t[:],
                        idx128[:, icol : icol + n_idx // 16],
                        n_idx,
                        n_idx,
                        ROW,
                        single_packet=False,
                        queue_num=call_no % 2,
                    )
                    icol += n_idx // 16
                    call_no += 1
                    t2 = fold_call(g, nf, c)
                    if not taper and gi == ncalls - 1:
                        last_data = t2
                    else:
                        # PE accumulates fold cols into the psum chain
                        stop = gi == ncalls - (1 if taper else 2)
                        nc.tensor.matmul(out=ps[:, 0:512], lhsT=ident[:],
                                         rhs=t2[:, 0:512],
                                         start=False, stop=stop)
                        nc.tensor.matmul(out=ps[:, 512:ROW], lhsT=ident[:],
                                         rhs=t2[:, 512:ROW],
                                         start=False, stop=stop)

                if taper:
                    # final (1,1) pair as half-row calls so the last SWDGE
                    # ring blocks are 1280 B/row: interleave q0/q1 halves to
                    # keep strict queue alternation.
                    gt = gath.tile([P, GMAX, ROW], F32, tag="g")
                    for half in range(2):
                        for i in range(2):
                            nc.gpsimd.dma_gather(
                                gt[:, i, half * HROW : (half + 1) * HROW]
                                    .rearrange("p (a e) -> p a e", a=1),
                                vaug_t[:, half * HROW : (half + 1) * HROW],
                                idx128[:, icol + i * 8 : icol + i * 8 + 8],
                                P,
                                P,
                                HROW,
                                elem_step=ROW,
                                single_packet=False,
                                queue_num=call_no % 2,
                            )
                            call_no += 1
                    tsum = small.tile([P, ROW], F32, tag="t2", bufs=4)
                    nc.vector.tensor_add(tsum[:], gt[:, 0, :], gt[:, 1, :])
                    last_data = tsum

                # fld = psum chain + last col (single PSUM read on DVE)
                fld = fold.tile([P, ROW], F32, tag="fld")
                nc.vector.tensor_add(fld[:], last_data[:], ps[:])

                # --- FM identity epilogue for this chunk ---
                blk = fld[:, 0:VCOLS]             # [128, 624] = field_f
                sq = fold.tile([P, VCOLS], F32, tag="sq")
                q = small.tile([P, 1], F32, tag="q")
                nc.scalar.activation(
                    sq[:], blk, mybir.ActivationFunctionType.Square,
                    accum_out=q[:],
                )
                # s = sum over the 39 fields: strided reduce of [128,16,39]
                st = fold.tile([P, 16], F32, tag="st")
                blk_kf = blk.rearrange("p (f k) -> p k f", k=16)
                nc.vector.tensor_reduce(
                    out=st[:], in_=blk_kf, op=mybir.AluOpType.add,
                    axis=mybir.AxisListType.X,
                )
                s2 = small.tile([P, 16], F32, tag="s2")
                snorm = small.tile([P, 1], F32, tag="snorm")
                nc.scalar.activation(
                    s2[:], st[:], mybir.ActivationFunctionType.Square,
                    accum_out=snorm[:],
                )
                diff = small.tile([P, 1], F32, tag="diff")
                nc.vector.tensor_tensor(
                    out=diff[:], in0=snorm[:], in1=q[:],
                    op=mybir.AluOpType.subtract,
                )
                # out = 0.5*diff + (w-sum incl. w0 and dense first-order)
                nc.scalar.activation(
                    res[:, c : c + 1],
                    diff[:],
                    mybir.ActivationFunctionType.Identity,
                    bias=fld[:, VCOLS : VCOLS + 1],
                    scale=0.5,
                )

            # transpose [128,4] -> [4,128] on PE so the store is 4 packets
            pst = psum.tile([SCHUNKS, P], F32, tag="pst", bufs=1)
            nc.tensor.transpose(pst[:], res[:], ident[:])
            res_t = main.tile([SCHUNKS, P], F32)
            nc.scalar.copy(res_t[:], pst[:])
            nc.sync.dma_start(out_t[:], res_t[:])

    nc.compile()
    return nc


def prep_inputs(dense_inputs, sparse_inputs, w0, w, v):
    """Host-side shard/pack: build per-core in_maps."""
    dense = np.asarray(dense_inputs, np.float32)
    sparse = np.asarray(sparse_inputs)
    w0 = np.asarray(w0, np.float32)
    w = np.asarray(w, np.float32)
    v = np.asarray(v, np.float32)

    vaug = np.zeros((FEATURE_NUM, ROW), np.float32)
    vaug[:, :VCOLS] = v.reshape(FEATURE_NUM, VCOLS)
    vaug[:, VCOLS] = w[:, 0]
    # fold w0 into sparse table 0 (each sample hits it exactly once)
    vaug[N_DENSE : N_DENSE + FEAT_PER_SPARSE, VCOLS] += w0[0]

    offs = N_DENSE + FEAT_PER_SPARSE * np.arange(N_SPARSE, dtype=np.int64)
    gidx = (sparse.astype(np.int64) + offs[None, :]).astype(np.int16)  # [B, 26]

    in_maps = []
    for core in range(N_CORES):
        sl = slice(core * BC, (core + 1) * BC)
        dt = np.ascontiguousarray(dense[sl].T)          # [13, 512]
        idxc = gidx[sl]                                 # [512, 26]
        buf = np.zeros((16, IDX_COLS), np.int16)        # wrapped 16-row pattern
        off_c = 0
        for c in range(SCHUNKS):
            rows = idxc[c * P : (c + 1) * P]            # [128, 26]
            fbase = 0
            for nf in FGROUPS_ALL[c]:
                n = nf * P
                # call order: i = f_local*128 + p  ->  row idx[p, fbase+f]
                seg = np.ascontiguousarray(
                    rows[:, fbase : fbase + nf].T
                ).reshape(-1)                           # [nf*128]
                buf[:, off_c : off_c + n // 16] = seg.reshape(n // 16, 16).T
                fbase += nf
                off_c += n // 16
        in_maps.append({"vaug": vaug, "dense_t": dt,
                        "idxs": np.tile(buf, (2, 1)),
                        "ident": np.eye(P, dtype=np.float32)})
    return in_maps


_NC_CACHE = None


def kernel(dense_inputs, sparse_inputs, w0, w, v):
    global _NC_CACHE
    if _NC_CACHE is None:
        _NC_CACHE = build_program()
    nc = _NC_CACHE
    in_maps = prep_inputs(dense_inputs, sparse_inputs, w0, w, v)
    res = run_bass_kernel_spmd(nc, in_maps, core_ids=list(range(N_CORES)))
    outs = []
    for r in res.results:
        o = r["out"]                                    # [4, 128]
        outs.append(np.ascontiguousarray(o).reshape(BC, 1))
    return np.concatenate(outs, axis=0).astype(np.float32)
